# revision 14
# baseline (speedup 1.0000x reference)
"""Trainium2 Bass kernel for nn_BasicRNN (2-layer LSTM, H=32, S=64, B=8192).

Strategy: pure data parallel over 8 cores (1024 batch each). T-layout tiles
[128 partitions = 4 groups x 32 features, free = batch-within-group]. Each
gate's projection is ONE full-width matmul with a block-diagonal lhsT
(scaled W_gate^T repeated on the diagonal), so matmul cost is 1/4 of the
per-group diagonal-subarray scheme.

The per-core batch is split into two halves (free columns 0:128 / 128:256 of
each group) that form two independent recurrence chains; their instruction
streams interleave so engine work of one half hides the serial-chain latency
of the other.

Per half-step: PSUM tile PALL [128, 512] = [f|i|o|g] x 128 in one PSUM bank.
A bias matmul (lhsT = per-gate biases, rhs = 0/1 selector) initializes PALL
(start=True), input and recurrent projections accumulate, then a single
tanh covers all four gates. Sigmoids use sigmoid(x) = (1 + tanh(x/2))/2 with
the 1/2 folded into weights/biases, and states are doubled (c* = 2c,
h* = 2h):
    u = (tf + 1) * c*          v = (ti + 1) * tanh_g
    c* = 0.5*u + v             h* = (to + 1) * tanh(0.5 * c*)
All weight scalings (tanh-trick 0.5 on f/i/o rows, 0.5 for consuming doubled
h*, 0.5 on w_out) are folded into the host-prepped weights.
"""
import sys
sys.path.insert(0, '/opt/trn_rl_repo')

import numpy as np

import concourse.bacc as bacc
import concourse.tile as tile
from concourse import mybir
from concourse.bass_utils import run_bass_kernel_spmd

F32 = mybir.dt.float32
F16 = mybir.dt.float16
TANH = mybir.ActivationFunctionType.Tanh
IDENT = mybir.ActivationFunctionType.Identity
ADD = mybir.AluOpType.add
MULT = mybir.AluOpType.mult

B, S, NX, NSFC, H, NY = 8192, 64, 4, 5, 32, 1
NCORES = 8
BC = B // NCORES          # 1024 batch per core
NG = 4                    # groups per core
GB = BC // NG             # 256 batch per group
NH = 2                    # independent half-chains per core
GBH = GB // NH            # 128 batch per group per half
# gate order in PALL free blocks: f, i, g, o (tanh-trick scale 0.5 on f/i/o;
# o last: its matmul+tanh sit off the critical recurrence path)
GATES = [("f", H, 0.5), ("i", 0, 0.5), ("g", 2 * H, 1.0), ("o", 3 * H, 0.5)]

_CACHED = {}


def _prep_weights(inp):
    """Host-side weight staging. Returns dict of DRAM arrays (shared by all
    cores)."""
    w = {}

    def lhsT_tile(wmat, scale_fio, scale_g, krows):
        # tile [128, 4*128]: per gate gi a [128,128] block-diagonal lhsT
        # (diag block g = scaled W_gate^T restricted to krows) so ONE matmul
        # per gate covers all 4 groups: out[32g+i,b] = sum_j W[j,i]*rhs[32g+j,b]
        t = np.zeros((128, 4 * 128), np.float32)
        for gi, (nm_g, r0, trick) in enumerate(GATES):
            s = (scale_g if nm_g == "g" else scale_fio)
            blk = (wmat[r0:r0 + H] * s).T.astype(np.float32)  # [K, 32]
            for gb in range(NG):
                t[32 * gb:32 * gb + krows,
                  128 * gi + 32 * gb:128 * gi + 32 * gb + 32] = blk[:krows]
        return t

    # layer 1: x is true scale; h1* is doubled.
    w["WX1"] = lhsT_tile(inp["w_ih1"], 0.5, 1.0, NX).astype(np.float16)
    w["WL1"] = lhsT_tile(inp["w_hh1"], 0.25, 0.5, H).astype(np.float16)
    # layer 2: input h1* doubled, h2* doubled.
    w["WX2"] = lhsT_tile(inp["w_ih2"], 0.25, 0.5, H).astype(np.float16)
    w["WL2"] = lhsT_tile(inp["w_hh2"], 0.25, 0.5, H).astype(np.float16)

    # bias-matmul lhsT [4, 2*128]: col block per layer; row k = gate k's
    # (b_ih+b_hh)*trick at T-layout partition p (feature p%32).
    bmm = np.zeros((4, 2 * 128), np.float32)
    for li, (bi, bh) in enumerate([(inp["b_ih1"], inp["b_hh1"]),
                                   (inp["b_ih2"], inp["b_hh2"])]):
        btot = bi + bh
        for gi, (_, r0, trick) in enumerate(GATES):
            vals = btot[r0:r0 + H] * trick  # [32]
            for gb in range(NG):
                bmm[gi, 128 * li + 32 * gb:128 * li + 32 * gb + 32] = vals
    w["BMM"] = bmm.astype(np.float16)

    # selector rhs [4, 4*GBH]: row k = 1 on gate k's free block
    sel = np.zeros((4, 4 * GBH), np.float32)
    for gi in range(4):
        sel[gi, GBH * gi:GBH * (gi + 1)] = 1.0
    w["SEL"] = sel.astype(np.float16)

    # sfc weights [8, 64]: rows 0:5 = [w_sfc1.T | w_sfc2.T]
    ws = np.zeros((8, 64), np.float32)
    ws[:NSFC, 0:32] = inp["w_sfc1"].T
    ws[:NSFC, 32:64] = inp["w_sfc2"].T
    w["WSFC"] = ws
    bs = np.zeros((128, 2), np.float32)
    for gb in range(NG):
        bs[32 * gb:32 * gb + 32, 0] = inp["b_sfc1"]
        bs[32 * gb:32 * gb + 32, 1] = inp["b_sfc2"]
    w["BSFC"] = bs

    # output weights [128, 1]: block g = (w_out * 0.5).T
    wo = np.zeros((128, 1), np.float32)
    for gb in range(NG):
        wo[32 * gb:32 * gb + 32, 0] = inp["w_out"][0] * 0.5
    w["WOUT"] = wo.astype(np.float16)
    w["BOUT"] = np.full((128, 1), float(inp["b_out"][0]), np.float32)
    return w


def build_program(n_steps=S, trace_sim=False, reps=0):
    nc = bacc.Bacc()
    d = {}
    # x preloaded wholesale: [NG, NX, S*GB]; X[32g+q, s*GB+n] = x_rev[g,q,s,n]
    d["xs"] = nc.declare_dram_parameter("xs", [NG, NX, n_steps * GB], F16,
                                        isOutput=False)
    d["sfcT"] = nc.declare_dram_parameter("sfcT", [8, BC], F32, isOutput=False)
    F16W = {"WX1", "WL1", "WX2", "WL2", "BMM", "SEL", "WOUT"}
    for nm, shape in [("WX1", [128, 512]), ("WL1", [128, 512]),
                      ("WX2", [128, 512]), ("WL2", [128, 512]),
                      ("BMM", [4, 256]), ("SEL", [4, 4 * GBH]),
                      ("WSFC", [8, 64]), ("BSFC", [128, 2]),
                      ("WOUT", [128, 1]), ("BOUT", [128, 1])]:
        d[nm] = nc.declare_dram_parameter(nm, shape,
                                          F16 if nm in F16W else F32,
                                          isOutput=False)
    y_out = nc.declare_dram_parameter("y", [NG, n_steps * GB], F32, isOutput=True)

    NS1 = n_steps + 1

    with tile.TileContext(nc, trace_sim=trace_sim) as tc:
        with tc.tile_pool(name="wpool", bufs=1) as wpool, \
             tc.tile_pool(name="big", bufs=1) as big, \
             tc.tile_pool(name="state", bufs=1) as state, \
             tc.tile_pool(name="work", bufs=2) as work, \
             tc.tile_pool(name="yp", bufs=2) as yp, \
             tc.tile_pool(name="psA", bufs=2, space="PSUM") as psA, \
             tc.tile_pool(name="psY", bufs=1, space="PSUM") as psY:

            # ---- stage weights ----
            W = {}
            for nm, shape in [("WX1", [128, 512]), ("WL1", [128, 512]),
                              ("WX2", [128, 512]), ("WL2", [128, 512]),
                              ("BMM", [4, 256]), ("SEL", [4, 4 * GBH]),
                              ("WSFC", [8, 64]), ("BSFC", [128, 2]),
                              ("WOUT", [128, 1]), ("BOUT", [128, 1])]:
                t = wpool.tile(shape, F16 if nm in F16W else F32, tag=nm)
                nc.sync.dma_start(t[:], d[nm][:])
                W[nm] = t
            sfcT = wpool.tile([8, BC], F32, tag="sfcT")
            nc.sync.dma_start(sfcT[:], d["sfcT"][:])
            # all-steps x tile; unused partition rows must be zero (the
            # full-width matmul reads them; 0-weight * NaN garbage = NaN).
            X = wpool.tile([128, n_steps * GB], F16, tag="X")
            nc.vector.memset(X[:], 0.0)
            for g in range(NG):
                nc.sync.dma_start(X[32 * g:32 * g + NX, :], d["xs"][g])

            # ---- big state storage ----
            h1_all = big.tile([128, NS1 * GB], F16, tag="h1_all")
            h2_all = big.tile([128, NS1 * GB], F16, tag="h2_all")
            # doubled cell state per half
            C = [state.tile([128, GBH], F32, tag=f"C{hf}", name=f"C{hf}")
                 for hf in range(NH)]

            # ---- init: h0/c0 from surface MLPs ----
            ph = psY.tile([128, 1024], F32, tag="PY")
            for g in range(NG):
                nc.tensor.matmul(ph[32 * g:32 * g + 32, 0:GB],
                                 W["WSFC"][0:NSFC, 0:32],
                                 sfcT[0:NSFC, GB * g:GB * (g + 1)],
                                 start=True, stop=True, tile_position=(0, 32 * g))
                nc.tensor.matmul(ph[32 * g:32 * g + 32, GB:2 * GB],
                                 W["WSFC"][0:NSFC, 32:64],
                                 sfcT[0:NSFC, GB * g:GB * (g + 1)],
                                 start=True, stop=True, tile_position=(0, 32 * g))
            t0 = work.tile([128, GB], F32, tag="t0")
            nc.scalar.activation(t0[:], ph[:, 0:GB], TANH, bias=W["BSFC"][:, 0:1])
            # h0* = 2*tanh(...)  stored at h1_all slice n_steps
            nc.vector.tensor_scalar_mul(
                h1_all[:, n_steps * GB:(n_steps + 1) * GB], t0[:], 2.0)
            t0b = work.tile([128, GB], F32, tag="t0")
            nc.scalar.activation(t0b[:], ph[:, GB:2 * GB], TANH, bias=W["BSFC"][:, 1:2])
            for hf in range(NH):
                nc.vector.tensor_scalar_mul(
                    C[hf][:], t0b[:, GBH * hf:GBH * (hf + 1)], 2.0)
            # layer-2 zero init state
            nc.vector.memset(h2_all[:, 0:GB], 0.0)

            # ---- the two sequential LSTM layers, two staggered half-chains ----
            def scan_body(iv=None):
                for layer in (1, 2):
                    WX = W["WX1"] if layer == 1 else W["WX2"]
                    WL = W["WL1"] if layer == 1 else W["WL2"]
                    bcol = 0 if layer == 1 else 128
                    if layer == 2:
                        for hf in range(NH):
                            nc.vector.memset(C[hf][:], 0.0)

                    def emit_proj(k, hf):
                        """Bias + input-projection matmuls for step k, half hf
                        (independent of the recurrence). One PSUM bank."""
                        pall = psA.tile([128, 4 * GBH], F32, tag=f"PA{hf}", name=f"PA{hf}")
                        nc.tensor.matmul(pall[:], W["BMM"][:, bcol:bcol + 128],
                                         W["SEL"][:], start=True, stop=False)
                        if layer == 1:
                            rhs = X[:, k * GB + GBH * hf:k * GB + GBH * (hf + 1)]
                        else:
                            rhs = h1_all[:, k * GB + GBH * hf:k * GB + GBH * (hf + 1)]
                        for gi in range(4):
                            nc.tensor.matmul(pall[:, GBH * gi:GBH * (gi + 1)],
                                             WX[:, 128 * gi:128 * gi + 128],
                                             rhs, start=False, stop=False)
                        return pall

                    nxt = [emit_proj(0, hf) for hf in range(NH)]
                    for k in range(n_steps):
                        # storage indices (h1 stored time-reversed; h2 forward)
                        if layer == 1:
                            rhs_idx, out_idx, hall = n_steps - k, n_steps - 1 - k, h1_all
                        else:
                            rhs_idx, out_idx, hall = k, k + 1, h2_all

                        cur = nxt
                        # recurrent matmuls: contiguous accumulation group per
                        # half; o emitted last so tanh(f,i,g) fires early
                        for hf in range(NH):
                            for gi in range(4):
                                nc.tensor.matmul(
                                    cur[hf][:, GBH * gi:GBH * (gi + 1)],
                                    WL[:, 128 * gi:128 * gi + 128],
                                    hall[:, rhs_idx * GB + GBH * hf:
                                         rhs_idx * GB + GBH * (hf + 1)],
                                    start=False, stop=(gi == 3))
                        TS = []
                        for hf in range(NH):
                            ts = work.tile([128, 4 * GBH], F16, tag=f"TS{hf}", name=f"TS{hf}")
                            nc.scalar.activation(ts[:, 0:3 * GBH],
                                                 cur[hf][:, 0:3 * GBH], TANH)
                            TS.append(ts)
                        for hf in range(NH):
                            nc.scalar.activation(TS[hf][:, 3 * GBH:4 * GBH],
                                                 cur[hf][:, 3 * GBH:4 * GBH], TANH)
                        # next step's projections run in the chain shadow
                        if k + 1 < n_steps:
                            nxt = [emit_proj(k + 1, hf) for hf in range(NH)]
                        UV = []
                        for hf in range(NH):
                            u = work.tile([128, GBH], F32, tag=f"U{hf}", name=f"U{hf}")
                            v = work.tile([128, GBH], F32, tag=f"V{hf}", name=f"V{hf}")
                            nc.vector.scalar_tensor_tensor(
                                u[:], TS[hf][:, 0:GBH], 1.0, C[hf][:], ADD, MULT)
                            nc.vector.scalar_tensor_tensor(
                                v[:], TS[hf][:, GBH:2 * GBH], 1.0,
                                TS[hf][:, 2 * GBH:3 * GBH], ADD, MULT)
                            UV.append((u, v))
                        for hf in range(NH):
                            nc.vector.scalar_tensor_tensor(
                                C[hf][:], UV[hf][0][:], 0.5, UV[hf][1][:],
                                MULT, ADD)
                        TC = []
                        for hf in range(NH):
                            tc_t = work.tile([128, GBH], F32, tag=f"TC{hf}", name=f"TC{hf}")
                            nc.scalar.activation(tc_t[:], C[hf][:], TANH, scale=0.5)
                            TC.append(tc_t)
                        for hf in range(NH):
                            nc.vector.scalar_tensor_tensor(
                                hall[:, out_idx * GB + GBH * hf:
                                     out_idx * GB + GBH * (hf + 1)],
                                TS[hf][:, 3 * GBH:4 * GBH], 1.0, TC[hf][:],
                                ADD, MULT)

            if reps:
                with tc.For_i(0, reps, 1) as iv:
                    scan_body(iv)
            else:
                scan_body()

            # ---- output projection: y = h2* @ (w_out/2).T + b_out ----
            YCH = 1024                       # free elems per chunk
            total = n_steps * GB
            nch = total // YCH
            for ci in range(nch):
                py = psY.tile([128, 1024], F32, tag="PY")
                for g in range(NG):
                    for j in range(YCH // 512):
                        off = GB + ci * YCH + j * 512
                        nc.tensor.matmul(py[32 * g:32 * g + 1, j * 512:(j + 1) * 512],
                                         W["WOUT"][32 * g:32 * g + 32, 0:1],
                                         h2_all[32 * g:32 * g + 32, off:off + 512],
                                         start=True, stop=True,
                                         tile_position=(32 * g, 32 * g))
                ysb = yp.tile([128, YCH], F32, tag="ysb")
                nc.scalar.activation(ysb[:], py[:], IDENT, bias=W["BOUT"][:, 0:1])
                for g in range(NG):
                    nc.sync.dma_start(y_out[g, ci * YCH:(ci + 1) * YCH],
                                      ysb[32 * g:32 * g + 1, :])
    nc.finalize()
    return nc


def _prep_core_inputs(inputs, wts, c):
    x = inputs["inputs_main"]          # [B, S, NX]
    sfc = inputs["inputs_sfc"]         # [B, NSFC]
    xs_c = x[c * BC:(c + 1) * BC]          # [BC, S, NX]
    sfc_c = sfc[c * BC:(c + 1) * BC]       # [BC, NSFC]
    xr = xs_c[:, ::-1, :]                  # time reversed
    # xs[g, q, s*GB+n] = x_rev[256g+n, s, q]
    xs_arr = np.ascontiguousarray(
        xr.reshape(NG, GB, S, NX).transpose(0, 3, 2, 1)).reshape(NG, NX, S * GB)
    sfcT = np.zeros((8, BC), np.float32)
    sfcT[:NSFC] = sfc_c.T
    m = {"xs": xs_arr.astype(np.float16), "sfcT": sfcT}
    m.update(wts)
    return m


def kernel(**inputs):
    inputs = {k: np.asarray(v) for k, v in inputs.items()}
    if "nc" not in _CACHED:
        _CACHED["nc"] = build_program(S)
    nc = _CACHED["nc"]

    wts = _prep_weights(inputs)
    in_maps = [_prep_core_inputs(inputs, wts, c) for c in range(NCORES)]

    res = run_bass_kernel_spmd(nc, in_maps, list(range(NCORES)))

    y = np.empty((B, S, NY), np.float32)
    for c in range(NCORES):
        yc = res.results[c]["y"]               # [NG, S*GB]
        yc = yc.reshape(NG, S, GB).transpose(0, 2, 1)   # [NG, GB, S]
        y[c * BC:(c + 1) * BC, :, 0] = yc.reshape(BC, S)
    return y


# revision 15
# speedup vs baseline: 2.4923x; 2.4923x over previous
"""Trainium2 Bass kernel for nn_BasicRNN (2-layer LSTM, H=32, S=64, B=8192).

Strategy: pure data parallel over 8 cores (1024 batch each). T-layout tiles
[128 partitions = 4 groups x 32 features, free = batch-within-group]. Each
gate's projection is ONE full-width matmul with a block-diagonal lhsT
(scaled W_gate^T repeated on the diagonal), so matmul cost is 1/4 of the
per-group diagonal-subarray scheme.

The per-core batch is split into two halves (free columns 0:128 / 128:256 of
each group) that form two independent recurrence chains; their instruction
streams interleave so engine work of one half hides the serial-chain latency
of the other.

Per half-step: PSUM tile PALL [128, 512] = [f|i|o|g] x 128 in one PSUM bank.
A bias matmul (lhsT = per-gate biases, rhs = 0/1 selector) initializes PALL
(start=True), input and recurrent projections accumulate, then a single
tanh covers all four gates. Sigmoids use sigmoid(x) = (1 + tanh(x/2))/2 with
the 1/2 folded into weights/biases, and states are doubled (c* = 2c,
h* = 2h):
    u = (tf + 1) * c*          v = (ti + 1) * tanh_g
    c* = 0.5*u + v             h* = (to + 1) * tanh(0.5 * c*)
All weight scalings (tanh-trick 0.5 on f/i/o rows, 0.5 for consuming doubled
h*, 0.5 on w_out) are folded into the host-prepped weights.
"""
import sys
sys.path.insert(0, '/opt/trn_rl_repo')

import numpy as np

import concourse.bacc as bacc
import concourse.tile as tile
from concourse import mybir
from concourse.bass_utils import run_bass_kernel_spmd

F32 = mybir.dt.float32
F16 = mybir.dt.float16
TANH = mybir.ActivationFunctionType.Tanh
IDENT = mybir.ActivationFunctionType.Identity
ADD = mybir.AluOpType.add
MULT = mybir.AluOpType.mult

B, S, NX, NSFC, H, NY = 8192, 64, 4, 5, 32, 1
NCORES = 8
BC = B // NCORES          # 1024 batch per core
NG = 4                    # groups per core
GB = BC // NG             # 256 batch per group
NH = 2                    # independent half-chains per core
GBH = GB // NH            # 128 batch per group per half
# gate order in PALL free blocks: f, i, o, g (tanh-trick scale 0.5 on f/i/o)
GATES = [("f", H, 0.5), ("i", 0, 0.5), ("o", 3 * H, 0.5), ("g", 2 * H, 1.0)]

_CACHED = {}


def _prep_weights(inp):
    """Host-side weight staging. Returns dict of DRAM arrays (shared by all
    cores)."""
    w = {}

    def lhsT_tile(wmat, scale_fio, scale_g, krows):
        # tile [128, 4*128]: per gate gi a [128,128] block-diagonal lhsT
        # (diag block g = scaled W_gate^T restricted to krows) so ONE matmul
        # per gate covers all 4 groups: out[32g+i,b] = sum_j W[j,i]*rhs[32g+j,b]
        t = np.zeros((128, 4 * 128), np.float32)
        for gi, (_, r0, trick) in enumerate(GATES):
            s = (scale_fio if gi < 3 else scale_g)
            blk = (wmat[r0:r0 + H] * s).T.astype(np.float32)  # [K, 32]
            for gb in range(NG):
                t[32 * gb:32 * gb + krows,
                  128 * gi + 32 * gb:128 * gi + 32 * gb + 32] = blk[:krows]
        return t

    # layer 1: x is true scale; h1* is doubled.
    w["WX1"] = lhsT_tile(inp["w_ih1"], 0.5, 1.0, NX).astype(np.float16)
    w["WL1"] = lhsT_tile(inp["w_hh1"], 0.25, 0.5, H).astype(np.float16)
    # layer 2: input h1* doubled, h2* doubled.
    w["WX2"] = lhsT_tile(inp["w_ih2"], 0.25, 0.5, H).astype(np.float16)
    w["WL2"] = lhsT_tile(inp["w_hh2"], 0.25, 0.5, H).astype(np.float16)

    # bias-matmul lhsT [4, 2*128]: col block per layer; row k = gate k's
    # (b_ih+b_hh)*trick at T-layout partition p (feature p%32).
    bmm = np.zeros((4, 2 * 128), np.float32)
    for li, (bi, bh) in enumerate([(inp["b_ih1"], inp["b_hh1"]),
                                   (inp["b_ih2"], inp["b_hh2"])]):
        btot = bi + bh
        for gi, (_, r0, trick) in enumerate(GATES):
            vals = btot[r0:r0 + H] * trick  # [32]
            for gb in range(NG):
                bmm[gi, 128 * li + 32 * gb:128 * li + 32 * gb + 32] = vals
    w["BMM"] = bmm.astype(np.float16)

    # selector rhs [4, 4*GBH]: row k = 1 on gate k's free block
    sel = np.zeros((4, 4 * GBH), np.float32)
    for gi in range(4):
        sel[gi, GBH * gi:GBH * (gi + 1)] = 1.0
    w["SEL"] = sel.astype(np.float16)

    # sfc weights [8, 64]: rows 0:5 = [w_sfc1.T | w_sfc2.T]
    ws = np.zeros((8, 64), np.float32)
    ws[:NSFC, 0:32] = inp["w_sfc1"].T
    ws[:NSFC, 32:64] = inp["w_sfc2"].T
    w["WSFC"] = ws
    bs = np.zeros((128, 2), np.float32)
    for gb in range(NG):
        bs[32 * gb:32 * gb + 32, 0] = inp["b_sfc1"]
        bs[32 * gb:32 * gb + 32, 1] = inp["b_sfc2"]
    w["BSFC"] = bs

    # output weights [128, 1]: block g = (w_out * 0.5).T
    wo = np.zeros((128, 1), np.float32)
    for gb in range(NG):
        wo[32 * gb:32 * gb + 32, 0] = inp["w_out"][0] * 0.5
    w["WOUT"] = wo.astype(np.float16)
    w["BOUT"] = np.full((128, 1), float(inp["b_out"][0]), np.float32)
    return w


def build_program(n_steps=S, trace_sim=False, reps=0):
    nc = bacc.Bacc()
    d = {}
    # x preloaded wholesale: [NG, NX, S*GB]; X[32g+q, s*GB+n] = x_rev[g,q,s,n]
    d["xs"] = nc.declare_dram_parameter("xs", [NG, NX, n_steps * GB], F16,
                                        isOutput=False)
    d["sfcT"] = nc.declare_dram_parameter("sfcT", [8, BC], F32, isOutput=False)
    F16W = {"WX1", "WL1", "WX2", "WL2", "BMM", "SEL", "WOUT"}
    for nm, shape in [("WX1", [128, 512]), ("WL1", [128, 512]),
                      ("WX2", [128, 512]), ("WL2", [128, 512]),
                      ("BMM", [4, 256]), ("SEL", [4, 4 * GBH]),
                      ("WSFC", [8, 64]), ("BSFC", [128, 2]),
                      ("WOUT", [128, 1]), ("BOUT", [128, 1])]:
        d[nm] = nc.declare_dram_parameter(nm, shape,
                                          F16 if nm in F16W else F32,
                                          isOutput=False)
    y_out = nc.declare_dram_parameter("y", [NG, n_steps * GB], F32, isOutput=True)

    NS1 = n_steps + 1

    with tile.TileContext(nc, trace_sim=trace_sim) as tc:
        with tc.tile_pool(name="wpool", bufs=1) as wpool, \
             tc.tile_pool(name="big", bufs=1) as big, \
             tc.tile_pool(name="state", bufs=1) as state, \
             tc.tile_pool(name="work", bufs=2) as work, \
             tc.tile_pool(name="yp", bufs=2) as yp, \
             tc.tile_pool(name="psA", bufs=2, space="PSUM") as psA, \
             tc.tile_pool(name="psY", bufs=1, space="PSUM") as psY:

            # ---- stage weights ----
            W = {}
            for nm, shape in [("WX1", [128, 512]), ("WL1", [128, 512]),
                              ("WX2", [128, 512]), ("WL2", [128, 512]),
                              ("BMM", [4, 256]), ("SEL", [4, 4 * GBH]),
                              ("WSFC", [8, 64]), ("BSFC", [128, 2]),
                              ("WOUT", [128, 1]), ("BOUT", [128, 1])]:
                t = wpool.tile(shape, F16 if nm in F16W else F32, tag=nm)
                nc.sync.dma_start(t[:], d[nm][:])
                W[nm] = t
            sfcT = wpool.tile([8, BC], F32, tag="sfcT")
            nc.sync.dma_start(sfcT[:], d["sfcT"][:])
            # all-steps x tile; unused partition rows must be zero (the
            # full-width matmul reads them; 0-weight * NaN garbage = NaN).
            X = wpool.tile([128, n_steps * GB], F16, tag="X")
            nc.vector.memset(X[:], 0.0)
            for g in range(NG):
                nc.sync.dma_start(X[32 * g:32 * g + NX, :], d["xs"][g])

            # ---- big state storage ----
            h1_all = big.tile([128, NS1 * GB], F16, tag="h1_all")
            h2_all = big.tile([128, NS1 * GB], F16, tag="h2_all")
            # doubled cell state per half
            C = [state.tile([128, GBH], F32, tag=f"C{hf}", name=f"C{hf}")
                 for hf in range(NH)]

            # ---- init: h0/c0 from surface MLPs ----
            ph = psY.tile([128, 1024], F32, tag="PY")
            for g in range(NG):
                nc.tensor.matmul(ph[32 * g:32 * g + 32, 0:GB],
                                 W["WSFC"][0:NSFC, 0:32],
                                 sfcT[0:NSFC, GB * g:GB * (g + 1)],
                                 start=True, stop=True, tile_position=(0, 32 * g))
                nc.tensor.matmul(ph[32 * g:32 * g + 32, GB:2 * GB],
                                 W["WSFC"][0:NSFC, 32:64],
                                 sfcT[0:NSFC, GB * g:GB * (g + 1)],
                                 start=True, stop=True, tile_position=(0, 32 * g))
            t0 = work.tile([128, GB], F32, tag="t0")
            nc.scalar.activation(t0[:], ph[:, 0:GB], TANH, bias=W["BSFC"][:, 0:1])
            # h0* = 2*tanh(...)  stored at h1_all slice n_steps
            nc.vector.tensor_scalar_mul(
                h1_all[:, n_steps * GB:(n_steps + 1) * GB], t0[:], 2.0)
            t0b = work.tile([128, GB], F32, tag="t0")
            nc.scalar.activation(t0b[:], ph[:, GB:2 * GB], TANH, bias=W["BSFC"][:, 1:2])
            for hf in range(NH):
                nc.vector.tensor_scalar_mul(
                    C[hf][:], t0b[:, GBH * hf:GBH * (hf + 1)], 2.0)
            # layer-2 zero init state
            nc.vector.memset(h2_all[:, 0:GB], 0.0)

            # ---- the two sequential LSTM layers, two staggered half-chains ----
            def scan_body(iv=None):
                for layer in (1, 2):
                    WX = W["WX1"] if layer == 1 else W["WX2"]
                    WL = W["WL1"] if layer == 1 else W["WL2"]
                    bcol = 0 if layer == 1 else 128
                    if layer == 2:
                        for hf in range(NH):
                            nc.vector.memset(C[hf][:], 0.0)

                    def emit_proj(k, hf):
                        """Bias + input-projection matmuls for step k, half hf
                        (independent of the recurrence). One PSUM bank."""
                        pall = psA.tile([128, 4 * GBH], F32, tag=f"PA{hf}", name=f"PA{hf}")
                        nc.tensor.matmul(pall[:], W["BMM"][:, bcol:bcol + 128],
                                         W["SEL"][:], start=True, stop=False)
                        if layer == 1:
                            rhs = X[:, k * GB + GBH * hf:k * GB + GBH * (hf + 1)]
                        else:
                            rhs = h1_all[:, k * GB + GBH * hf:k * GB + GBH * (hf + 1)]
                        for gi in range(4):
                            nc.tensor.matmul(pall[:, GBH * gi:GBH * (gi + 1)],
                                             WX[:, 128 * gi:128 * gi + 128],
                                             rhs, start=False, stop=False)
                        return pall

                    nxt = [emit_proj(0, hf) for hf in range(NH)]
                    for k in range(n_steps):
                        # storage indices (h1 stored time-reversed; h2 forward)
                        if layer == 1:
                            rhs_idx, out_idx, hall = n_steps - k, n_steps - 1 - k, h1_all
                        else:
                            rhs_idx, out_idx, hall = k, k + 1, h2_all

                        cur = nxt
                        for hf in range(NH):
                            for gi in range(4):
                                nc.tensor.matmul(
                                    cur[hf][:, GBH * gi:GBH * (gi + 1)],
                                    WL[:, 128 * gi:128 * gi + 128],
                                    hall[:, rhs_idx * GB + GBH * hf:
                                         rhs_idx * GB + GBH * (hf + 1)],
                                    start=False, stop=(gi == 3))
                        TS = []
                        for hf in range(NH):
                            ts = work.tile([128, 4 * GBH], F16, tag=f"TS{hf}", name=f"TS{hf}")
                            nc.scalar.activation(ts[:], cur[hf][:], TANH)
                            TS.append(ts)
                        # next step's projections run in the chain shadow
                        if k + 1 < n_steps:
                            nxt = [emit_proj(k + 1, hf) for hf in range(NH)]
                        UV = []
                        for hf in range(NH):
                            u = work.tile([128, GBH], F32, tag=f"U{hf}", name=f"U{hf}")
                            v = work.tile([128, GBH], F32, tag=f"V{hf}", name=f"V{hf}")
                            nc.vector.scalar_tensor_tensor(
                                u[:], TS[hf][:, 0:GBH], 1.0, C[hf][:], ADD, MULT)
                            nc.vector.scalar_tensor_tensor(
                                v[:], TS[hf][:, GBH:2 * GBH], 1.0,
                                TS[hf][:, 3 * GBH:4 * GBH], ADD, MULT)
                            UV.append((u, v))
                        for hf in range(NH):
                            nc.vector.scalar_tensor_tensor(
                                C[hf][:], UV[hf][0][:], 0.5, UV[hf][1][:],
                                MULT, ADD)
                        TC = []
                        for hf in range(NH):
                            tc_t = work.tile([128, GBH], F32, tag=f"TC{hf}", name=f"TC{hf}")
                            nc.scalar.activation(tc_t[:], C[hf][:], TANH, scale=0.5)
                            TC.append(tc_t)
                        for hf in range(NH):
                            nc.vector.scalar_tensor_tensor(
                                hall[:, out_idx * GB + GBH * hf:
                                     out_idx * GB + GBH * (hf + 1)],
                                TS[hf][:, 2 * GBH:3 * GBH], 1.0, TC[hf][:],
                                ADD, MULT)

            if reps:
                with tc.For_i(0, reps, 1) as iv:
                    scan_body(iv)
            else:
                scan_body()

            # ---- output projection: y = h2* @ (w_out/2).T + b_out ----
            YCH = 1024                       # free elems per chunk
            total = n_steps * GB
            nch = total // YCH
            for ci in range(nch):
                py = psY.tile([128, 1024], F32, tag="PY")
                for g in range(NG):
                    for j in range(YCH // 512):
                        off = GB + ci * YCH + j * 512
                        nc.tensor.matmul(py[32 * g:32 * g + 1, j * 512:(j + 1) * 512],
                                         W["WOUT"][32 * g:32 * g + 32, 0:1],
                                         h2_all[32 * g:32 * g + 32, off:off + 512],
                                         start=True, stop=True,
                                         tile_position=(32 * g, 32 * g))
                ysb = yp.tile([128, YCH], F32, tag="ysb")
                nc.scalar.activation(ysb[:], py[:], IDENT, bias=W["BOUT"][:, 0:1])
                for g in range(NG):
                    nc.sync.dma_start(y_out[g, ci * YCH:(ci + 1) * YCH],
                                      ysb[32 * g:32 * g + 1, :])
    nc.finalize()
    return nc


def _prep_core_inputs(inputs, wts, c):
    x = inputs["inputs_main"]          # [B, S, NX]
    sfc = inputs["inputs_sfc"]         # [B, NSFC]
    xs_c = x[c * BC:(c + 1) * BC]          # [BC, S, NX]
    sfc_c = sfc[c * BC:(c + 1) * BC]       # [BC, NSFC]
    xr = xs_c[:, ::-1, :]                  # time reversed
    # xs[g, q, s*GB+n] = x_rev[256g+n, s, q]
    xs_arr = np.ascontiguousarray(
        xr.reshape(NG, GB, S, NX).transpose(0, 3, 2, 1)).reshape(NG, NX, S * GB)
    sfcT = np.zeros((8, BC), np.float32)
    sfcT[:NSFC] = sfc_c.T
    m = {"xs": xs_arr.astype(np.float16), "sfcT": sfcT}
    m.update(wts)
    return m


def kernel(**inputs):
    inputs = {k: np.asarray(v) for k, v in inputs.items()}
    if "nc" not in _CACHED:
        _CACHED["nc"] = build_program(S)
    nc = _CACHED["nc"]

    wts = _prep_weights(inputs)
    in_maps = [_prep_core_inputs(inputs, wts, c) for c in range(NCORES)]

    res = run_bass_kernel_spmd(nc, in_maps, list(range(NCORES)))

    y = np.empty((B, S, NY), np.float32)
    for c in range(NCORES):
        yc = res.results[c]["y"]               # [NG, S*GB]
        yc = yc.reshape(NG, S, GB).transpose(0, 2, 1)   # [NG, GB, S]
        y[c * BC:(c + 1) * BC, :, 0] = yc.reshape(BC, S)
    return y
